# revision 9
# baseline (speedup 1.0000x reference)
"""Trainium2 Bass kernel for nn_CheckEmotion: embedding gather -> LSTM(128) -> linear(28).

Strategy (data-parallel over 8 NeuronCores, 64 batch rows each):
  - Embedding gather via ONE batched indirect DMA per 8192-token block
    (multi-column offset AP), table padded to 68 cols with col 64 == 1.0
    (the ones column folds the gate bias into the input projection matmul).
  - ONE blocked DMA-xbar transpose per block via 3D APs:
    xeT[e, j, t] = gath[t, j, e].
  - Input projections gx per 8-step window directly into PSUM
    (K=68 bf16 matmul, N=512; bias comes from the ones row).
  - Recurrence: per step 4 gate matmuls (bf16, K=128, N=64) accumulate onto
    gx in PSUM; sigmoid over [i,f,o] banks in one ACT op, tanh for g;
    fused [sig_i|sig_f] * [tanh_g|c_prev] DVE mul; h kept in bf16.
  - Final linear computed transposed ([28, 64] = w_lin @ h.T + b); host
    transposes back.
"""
import sys
import numpy as np
from contextlib import ExitStack

sys.path.insert(0, '/opt/trn_rl_repo')

import concourse.bass as bass
import concourse.tile as tile
from concourse import bacc, mybir
from concourse.bass_utils import run_bass_kernel_spmd

V, E, H, NCLS = 50257, 64, 128, 28
B, T = 512, 1024
NCORES = 8
BC = B // NCORES            # 64 batch rows per core
TOK = BC * T                # 65536 tokens per core
BLK = 8192                  # tokens per gather block
NBLK = TOK // BLK           # 8
WSTEPS = 8                  # steps per psum window
TPB = BLK // 128            # gather tiles per block (64)

F32 = mybir.dt.float32
BF16 = mybir.dt.bfloat16
I32 = mybir.dt.int32
EP = 128                    # padded embedding row: 64 emb + ones col (bias) + zeros

_NC_CACHE = {}


def build_nc():
    if 'nc' in _NC_CACHE:
        return _NC_CACHE['nc']
    nc = bacc.Bacc("TRN2", target_bir_lowering=False, debug=False)
    emb = nc.dram_tensor("emb", [V, EP], BF16, kind="ExternalInput")
    idx = nc.dram_tensor("idx", [128, TOK // 128], I32, kind="ExternalInput")
    wih = nc.dram_tensor("wih", [EP, 4 * H], BF16, kind="ExternalInput")  # lhsT, gate-major i,f,o,g; row 64 = bias
    whh = nc.dram_tensor("whh", [H, 4 * H], BF16, kind="ExternalInput")   # lhsT, gate-major i,f,o,g
    wlin = nc.dram_tensor("wlin", [H, NCLS], F32, kind="ExternalInput")   # lhsT
    blin = nc.dram_tensor("blin", [NCLS, 1], F32, kind="ExternalInput")
    out = nc.dram_tensor("out", [NCLS, BC], F32, kind="ExternalOutput")
    dbg_x = nc.dram_tensor("dbg_x", [EP, 256], BF16, kind="ExternalOutput")

    with tile.TileContext(nc) as tc, ExitStack() as ctx:
        singles = ctx.enter_context(tc.tile_pool(name="singles", bufs=1))
        gathp = ctx.enter_context(tc.tile_pool(name="gath", bufs=2))
        xep = ctx.enter_context(tc.tile_pool(name="xeT", bufs=2))
        psump = ctx.enter_context(tc.tile_pool(name="ps", bufs=2, space="PSUM"))
        gatep = ctx.enter_context(tc.tile_pool(name="gates", bufs=3))
        tmpp = ctx.enter_context(tc.tile_pool(name="tmp", bufs=4))
        statep = ctx.enter_context(tc.tile_pool(name="state", bufs=3))

        idx_sb = singles.tile([128, TOK // 128], I32)
        nc.sync.dma_start(out=idx_sb[:], in_=idx[:, :])
        wih_sb = singles.tile([EP, 4 * H], BF16)
        nc.sync.dma_start(out=wih_sb[:], in_=wih[:, :])
        whh_sb = singles.tile([H, 4 * H], BF16)
        nc.sync.dma_start(out=whh_sb[:], in_=whh[:, :])
        wlin_sb = singles.tile([H, NCLS], F32)
        nc.sync.dma_start(out=wlin_sb[:], in_=wlin[:, :])
        blin_sb = singles.tile([NCLS, 1], F32)
        nc.sync.dma_start(out=blin_sb[:], in_=blin[:, :])

        # Two interleaved chains (batch halves) so ACT/DVE/PE overlap across
        # chains while each chain's serial recurrence latency is hidden.
        NCH = 2
        CB = BC // NCH              # 32 batch cols per chain
        h_prev = []
        cpair_cur = []
        for X in range(NCH):
            hX = statep.tile([H, CB], BF16, tag=f"h{X}", name=f"h{X}")
            nc.vector.memset(hX[:], 0.0)
            h_prev.append(hX)
            # cpair_j holds [tanh(g_j) | c_{j-1}] for the fused pair multiply.
            cpX = statep.tile([H, 2, CB], F32, tag=f"cpair{X}", name=f"cpair{X}")
            nc.vector.memset(cpX[:, 1, :], 0.0)
            cpair_cur.append(cpX)

        for blk in range(NBLK):
            gath = gathp.tile([128, TPB, EP], BF16, tag="gath")
            for j in range(TPB):
                nc.gpsimd.indirect_dma_start(
                    out=gath[:, j, :],
                    out_offset=None,
                    in_=emb[:, :],
                    in_offset=bass.IndirectOffsetOnAxis(
                        ap=idx_sb[:, blk * TPB + j:blk * TPB + j + 1], axis=0),
                )
            # Blocked xbar transpose: xeT[e, j, t] = gath[t, j, e]
            xeT = xep.tile([EP, TPB, 128], BF16, tag="xeT")
            nc.sync.dma_start(
                out=xeT[:, :, :],
                in_=gath[:, :, :],
                transpose=True,
            )
            xeT2 = xeT.rearrange("e j t -> e (j t)")
            if blk == 0:
                nc.sync.dma_start(out=dbg_x[:, :], in_=xeT2[:, 0:256])
            for w in range(BLK // (WSTEPS * BC)):            # 16 windows per block
                ps = psump.tile([128, 4, 512], F32, tag="ps")
                for g in range(4):
                    nc.tensor.matmul(
                        out=ps[:, g, :],
                        lhsT=wih_sb[:, g * H:(g + 1) * H],
                        rhs=xeT2[:, w * 512:(w + 1) * 512],
                        start=True, stop=False, skip_group_check=True,
                    )
                for j in range(WSTEPS):
                    cs = [slice(j * BC + X * CB, j * BC + (X + 1) * CB)
                          for X in range(NCH)]
                    for X in range(NCH):
                        for g in range(4):
                            nc.tensor.matmul(
                                out=ps[:, g, cs[X]],
                                lhsT=whh_sb[:, g * H:(g + 1) * H],
                                rhs=h_prev[X][:, :],
                                start=False, stop=True, skip_group_check=True,
                            )
                    gsig = []
                    for X in range(NCH):
                        gs = gatep.tile([H, 3, CB], F32, tag=f"gsig{X}",
                                        name=f"gsig{X}")
                        nc.scalar.activation(
                            out=gs[:, :, :],
                            in_=ps[:, 0:3, cs[X]],
                            func=mybir.ActivationFunctionType.Sigmoid,
                        )
                        nc.scalar.activation(
                            out=cpair_cur[X][:, 0, :],
                            in_=ps[:, 3, cs[X]],
                            func=mybir.ActivationFunctionType.Tanh,
                        )
                        gsig.append(gs)
                    cpair_next = []
                    for X in range(NCH):
                        # t12 = [sig_i | sig_f] * [tanh_g | c_prev]
                        t12 = tmpp.tile([H, 2, CB], F32, tag=f"t12{X}",
                                        name=f"t12{X}")
                        nc.vector.tensor_mul(t12[:, :, :], gsig[X][:, 0:2, :],
                                             cpair_cur[X][:, :, :])
                        cpn = statep.tile([H, 2, CB], F32, tag=f"cpair{X}",
                                          name=f"cpairn{X}")
                        nc.vector.tensor_add(cpn[:, 1, :], t12[:, 0, :],
                                             t12[:, 1, :])
                        cpair_next.append(cpn)
                    tc_t = []
                    for X in range(NCH):
                        tcX = tmpp.tile([H, CB], F32, tag=f"tc{X}",
                                        name=f"tc{X}")
                        nc.scalar.activation(
                            out=tcX[:], in_=cpair_next[X][:, 1, :],
                            func=mybir.ActivationFunctionType.Tanh,
                        )
                        tc_t.append(tcX)
                    for X in range(NCH):
                        h_new = statep.tile([H, CB], BF16, tag=f"h{X}",
                                            name=f"hn{X}")
                        nc.vector.tensor_mul(h_new[:], gsig[X][:, 2, :],
                                             tc_t[X][:])
                        h_prev[X] = h_new
                        cpair_cur[X] = cpair_next[X]

        ps_f = psump.tile([NCLS, BC], F32, tag="ps")
        wlin_bf = tmpp.tile([H, NCLS], BF16, tag="wlinb")
        nc.scalar.copy(out=wlin_bf[:], in_=wlin_sb[:])
        for X in range(NCH):
            nc.tensor.matmul(out=ps_f[:, X * CB:(X + 1) * CB],
                             lhsT=wlin_bf[:, :], rhs=h_prev[X][:, :],
                             start=True, stop=True, skip_group_check=True)
        out_sb = tmpp.tile([NCLS, BC], F32, tag="outsb")
        nc.scalar.activation(
            out=out_sb[:, :], in_=ps_f[:, :],
            func=mybir.ActivationFunctionType.Identity,
            bias=blin_sb[:, :],
        )
        nc.sync.dma_start(out=out[:, :], in_=out_sb[:, :])

    nc.compile()
    _NC_CACHE['nc'] = nc
    return nc


def prep_inputs(x, emb_table, w_ih, w_hh, b_ih, b_hh, w_lin, b_lin):
    """Host-side prep: shard batch, reorder weights gate-major [i,f,o,g] as lhsT."""
    import ml_dtypes
    x = np.asarray(x)
    emb_f32 = np.asarray(emb_table, dtype=np.float32)
    emb_pad = np.zeros((V, EP), dtype=ml_dtypes.bfloat16)
    emb_pad[:, :E] = emb_f32.astype(ml_dtypes.bfloat16)
    emb_pad[:, E] = 1.0                                     # ones col -> bias row of wih
    w_ih = np.asarray(w_ih, dtype=np.float32)
    w_hh = np.asarray(w_hh, dtype=np.float32)
    bias = np.asarray(b_ih, dtype=np.float32) + np.asarray(b_hh, dtype=np.float32)
    w_lin = np.asarray(w_lin, dtype=np.float32)
    b_lin = np.asarray(b_lin, dtype=np.float32)

    # torch gate order i,f,g,o -> our psum order i,f,o,g
    perm = [0, 1, 3, 2]
    wih_g = w_ih.reshape(4, H, E)[perm]                     # [4, H, E]
    whh_g = w_hh.reshape(4, H, H)[perm]
    bias_g = bias.reshape(4, H)[perm]
    wih_lhsT = np.zeros((EP, 4 * H), dtype=ml_dtypes.bfloat16)
    wih_lhsT[:E] = np.ascontiguousarray(
        np.transpose(wih_g, (2, 0, 1)).reshape(E, 4 * H)).astype(
            ml_dtypes.bfloat16)                             # [E, 4H] (gate-major cols)
    wih_lhsT[E] = bias_g.reshape(4 * H).astype(ml_dtypes.bfloat16)
    whh_lhsT = np.ascontiguousarray(
        np.transpose(whh_g, (2, 0, 1)).reshape(H, 4 * H)).astype(
            ml_dtypes.bfloat16)
    wlin_lhsT = np.ascontiguousarray(w_lin.T)               # [H, NCLS]
    blin = np.ascontiguousarray(b_lin.reshape(NCLS, 1))

    in_maps = []
    for c in range(NCORES):
        xc = x[c * BC:(c + 1) * BC]                         # [BC, T]
        toks = np.ascontiguousarray(xc.T).reshape(-1)       # t-major: k = t*BC + b
        idx_host = np.ascontiguousarray(
            toks.astype(np.int32).reshape(TOK // 128, 128).T)  # [128, TOK/128]
        in_maps.append({
            "emb": emb_pad,
            "idx": idx_host,
            "wih": wih_lhsT,
            "whh": whh_lhsT,
            "wlin": wlin_lhsT,
            "blin": blin,
        })
    return in_maps


def run(inputs, trace=False):
    nc = build_nc()
    in_maps = prep_inputs(**inputs)
    res = run_bass_kernel_spmd(nc, in_maps, core_ids=list(range(NCORES)),
                               trace=trace)
    outs = [r["out"] for r in res.results]                  # each [NCLS, BC]
    full = np.concatenate([o.T for o in outs], axis=0)      # [B, NCLS]
    return full.astype(np.float32), res


def kernel(**inputs):
    out, _ = run(inputs, trace=False)
    return out


# revision 12
# speedup vs baseline: 1.1463x; 1.1463x over previous
"""Trainium2 Bass kernel for nn_CheckEmotion: embedding gather -> LSTM(128) -> linear(28).

Strategy (data-parallel over 8 NeuronCores, 64 batch rows each):
  - Embedding gather via ONE batched indirect DMA per 8192-token block
    (multi-column offset AP), table padded to 68 cols with col 64 == 1.0
    (the ones column folds the gate bias into the input projection matmul).
  - ONE blocked DMA-xbar transpose per block via 3D APs:
    xeT[e, j, t] = gath[t, j, e].
  - Input projections gx per 8-step window directly into PSUM
    (K=68 bf16 matmul, N=512; bias comes from the ones row).
  - Recurrence: per step 4 gate matmuls (bf16, K=128, N=64) accumulate onto
    gx in PSUM; sigmoid over [i,f,o] banks in one ACT op, tanh for g;
    fused [sig_i|sig_f] * [tanh_g|c_prev] DVE mul; h kept in bf16.
  - Final linear computed transposed ([28, 64] = w_lin @ h.T + b); host
    transposes back.
"""
import sys
import numpy as np
from contextlib import ExitStack

sys.path.insert(0, '/opt/trn_rl_repo')

import concourse.bass as bass
import concourse.tile as tile
from concourse import bacc, mybir
from concourse.bass_utils import run_bass_kernel_spmd

V, E, H, NCLS = 50257, 64, 128, 28
B, T = 512, 1024
NCORES = 8
BC = B // NCORES            # 64 batch rows per core
TOK = BC * T                # 65536 tokens per core
BLK = 8192                  # tokens per gather block
NBLK = TOK // BLK           # 8
WSTEPS = 8                  # steps per psum window
TPB = BLK // 128            # gather tiles per block (64)

F32 = mybir.dt.float32
BF16 = mybir.dt.bfloat16
I32 = mybir.dt.int32
EP = 128                    # padded embedding row: 64 emb + ones col (bias) + zeros

_NC_CACHE = {}


def build_nc():
    if 'nc' in _NC_CACHE:
        return _NC_CACHE['nc']
    nc = bacc.Bacc("TRN2", target_bir_lowering=False, debug=False)
    emb = nc.dram_tensor("emb", [V, EP], BF16, kind="ExternalInput")
    idx = nc.dram_tensor("idx", [128, TOK // 128], I32, kind="ExternalInput")
    wih = nc.dram_tensor("wih", [EP, 4 * H], BF16, kind="ExternalInput")  # lhsT, gate-major i,f,o,g; row 64 = bias
    whh = nc.dram_tensor("whh", [H, 4 * H], BF16, kind="ExternalInput")   # lhsT, gate-major i,f,o,g
    wlin = nc.dram_tensor("wlin", [H, NCLS], F32, kind="ExternalInput")   # lhsT
    blin = nc.dram_tensor("blin", [NCLS, 1], F32, kind="ExternalInput")
    out = nc.dram_tensor("out", [NCLS, BC], F32, kind="ExternalOutput")
    dbg_x = nc.dram_tensor("dbg_x", [EP, 256], BF16, kind="ExternalOutput")

    with tile.TileContext(nc) as tc, ExitStack() as ctx:
        singles = ctx.enter_context(tc.tile_pool(name="singles", bufs=1))
        gathp = ctx.enter_context(tc.tile_pool(name="gath", bufs=2))
        xep = ctx.enter_context(tc.tile_pool(name="xeT", bufs=2))
        psump = ctx.enter_context(tc.tile_pool(name="ps", bufs=2, space="PSUM"))
        gatep = ctx.enter_context(tc.tile_pool(name="gates", bufs=3))
        tmpp = ctx.enter_context(tc.tile_pool(name="tmp", bufs=4))
        statep = ctx.enter_context(tc.tile_pool(name="state", bufs=3))

        idx_sb = singles.tile([128, TOK // 128], I32)
        nc.sync.dma_start(out=idx_sb[:], in_=idx[:, :])
        wih_sb = singles.tile([EP, 4 * H], BF16)
        nc.sync.dma_start(out=wih_sb[:], in_=wih[:, :])
        whh_sb = singles.tile([H, 4 * H], BF16)
        nc.sync.dma_start(out=whh_sb[:], in_=whh[:, :])
        wlin_sb = singles.tile([H, NCLS], F32)
        nc.sync.dma_start(out=wlin_sb[:], in_=wlin[:, :])
        blin_sb = singles.tile([NCLS, 1], F32)
        nc.sync.dma_start(out=blin_sb[:], in_=blin[:, :])

        h_prev = statep.tile([H, BC], BF16, tag="h")
        nc.vector.memset(h_prev[:], 0.0)
        # cpair_j holds [tanh(g_j) | c_{j-1}] for the fused pair multiply.
        cpair_cur = statep.tile([H, 2, BC], F32, tag="cpair")
        nc.vector.memset(cpair_cur[:, 1, :], 0.0)

        for blk in range(NBLK):
            gath = gathp.tile([128, TPB, EP], BF16, tag="gath")
            for j in range(TPB):
                nc.gpsimd.indirect_dma_start(
                    out=gath[:, j, :],
                    out_offset=None,
                    in_=emb[:, :],
                    in_offset=bass.IndirectOffsetOnAxis(
                        ap=idx_sb[:, blk * TPB + j:blk * TPB + j + 1], axis=0),
                )
            # Blocked xbar transpose: xeT[e, j, t] = gath[t, j, e]
            xeT = xep.tile([EP, TPB, 128], BF16, tag="xeT")
            nc.sync.dma_start(
                out=xeT[:, :, :],
                in_=gath[:, :, :],
                transpose=True,
            )
            xeT2 = xeT.rearrange("e j t -> e (j t)")
            if blk == 0:
                nc.sync.dma_start(out=dbg_x[:, :], in_=xeT2[:, 0:256])
            for w in range(BLK // (WSTEPS * BC)):            # 16 windows per block
                ps = psump.tile([128, 4, 512], F32, tag="ps")
                for g in range(4):
                    nc.tensor.matmul(
                        out=ps[:, g, :],
                        lhsT=wih_sb[:, g * H:(g + 1) * H],
                        rhs=xeT2[:, w * 512:(w + 1) * 512],
                        start=True, stop=False, skip_group_check=True,
                    )
                for j in range(WSTEPS):
                    cj = slice(j * BC, (j + 1) * BC)
                    # g-gate matmul FIRST so its tanh overlaps the i/f/o matmuls
                    for g in (3, 0, 1, 2):
                        nc.tensor.matmul(
                            out=ps[:, g, cj],
                            lhsT=whh_sb[:, g * H:(g + 1) * H],
                            rhs=h_prev[:, :],
                            start=False, stop=True, skip_group_check=True,
                        )
                    nc.scalar.activation(
                        out=cpair_cur[:, 0, :],
                        in_=ps[:, 3, cj],
                        func=mybir.ActivationFunctionType.Tanh,
                    )
                    gsig = gatep.tile([H, 3, BC], F32, tag="gsig")
                    nc.scalar.activation(
                        out=gsig[:, :, :],
                        in_=ps[:, 0:3, cj],
                        func=mybir.ActivationFunctionType.Sigmoid,
                    )
                    # t12 = [sig_i | sig_f] * [tanh_g | c_prev]
                    t12 = tmpp.tile([H, 2, BC], F32, tag="t12")
                    nc.vector.tensor_mul(t12[:, :, :], gsig[:, 0:2, :],
                                         cpair_cur[:, :, :])
                    cpair_next = statep.tile([H, 2, BC], F32, tag="cpair")
                    nc.vector.tensor_add(cpair_next[:, 1, :], t12[:, 0, :],
                                         t12[:, 1, :])
                    tc_t = tmpp.tile([H, BC], F32, tag="tc")
                    nc.scalar.activation(
                        out=tc_t[:], in_=cpair_next[:, 1, :],
                        func=mybir.ActivationFunctionType.Tanh,
                    )
                    h_new = statep.tile([H, BC], BF16, tag="h")
                    nc.vector.tensor_mul(h_new[:], gsig[:, 2, :], tc_t[:])
                    h_prev = h_new
                    cpair_cur = cpair_next

        ps_f = psump.tile([NCLS, BC], F32, tag="ps")
        wlin_bf = tmpp.tile([H, NCLS], BF16, tag="wlinb")
        nc.scalar.copy(out=wlin_bf[:], in_=wlin_sb[:])
        nc.tensor.matmul(out=ps_f[:, :], lhsT=wlin_bf[:, :], rhs=h_prev[:, :],
                         start=True, stop=True, skip_group_check=True)
        out_sb = tmpp.tile([NCLS, BC], F32, tag="outsb")
        nc.scalar.activation(
            out=out_sb[:, :], in_=ps_f[:, :],
            func=mybir.ActivationFunctionType.Identity,
            bias=blin_sb[:, :],
        )
        nc.sync.dma_start(out=out[:, :], in_=out_sb[:, :])

    nc.compile()
    _NC_CACHE['nc'] = nc
    return nc


def prep_inputs(x, emb_table, w_ih, w_hh, b_ih, b_hh, w_lin, b_lin):
    """Host-side prep: shard batch, reorder weights gate-major [i,f,o,g] as lhsT."""
    import ml_dtypes
    x = np.asarray(x)
    emb_f32 = np.asarray(emb_table, dtype=np.float32)
    emb_pad = np.zeros((V, EP), dtype=ml_dtypes.bfloat16)
    emb_pad[:, :E] = emb_f32.astype(ml_dtypes.bfloat16)
    emb_pad[:, E] = 1.0                                     # ones col -> bias row of wih
    w_ih = np.asarray(w_ih, dtype=np.float32)
    w_hh = np.asarray(w_hh, dtype=np.float32)
    bias = np.asarray(b_ih, dtype=np.float32) + np.asarray(b_hh, dtype=np.float32)
    w_lin = np.asarray(w_lin, dtype=np.float32)
    b_lin = np.asarray(b_lin, dtype=np.float32)

    # torch gate order i,f,g,o -> our psum order i,f,o,g
    perm = [0, 1, 3, 2]
    wih_g = w_ih.reshape(4, H, E)[perm]                     # [4, H, E]
    whh_g = w_hh.reshape(4, H, H)[perm]
    bias_g = bias.reshape(4, H)[perm]
    wih_lhsT = np.zeros((EP, 4 * H), dtype=ml_dtypes.bfloat16)
    wih_lhsT[:E] = np.ascontiguousarray(
        np.transpose(wih_g, (2, 0, 1)).reshape(E, 4 * H)).astype(
            ml_dtypes.bfloat16)                             # [E, 4H] (gate-major cols)
    wih_lhsT[E] = bias_g.reshape(4 * H).astype(ml_dtypes.bfloat16)
    whh_lhsT = np.ascontiguousarray(
        np.transpose(whh_g, (2, 0, 1)).reshape(H, 4 * H)).astype(
            ml_dtypes.bfloat16)
    wlin_lhsT = np.ascontiguousarray(w_lin.T)               # [H, NCLS]
    blin = np.ascontiguousarray(b_lin.reshape(NCLS, 1))

    in_maps = []
    for c in range(NCORES):
        xc = x[c * BC:(c + 1) * BC]                         # [BC, T]
        toks = np.ascontiguousarray(xc.T).reshape(-1)       # t-major: k = t*BC + b
        idx_host = np.ascontiguousarray(
            toks.astype(np.int32).reshape(TOK // 128, 128).T)  # [128, TOK/128]
        in_maps.append({
            "emb": emb_pad,
            "idx": idx_host,
            "wih": wih_lhsT,
            "whh": whh_lhsT,
            "wlin": wlin_lhsT,
            "blin": blin,
        })
    return in_maps


def run(inputs, trace=False):
    nc = build_nc()
    in_maps = prep_inputs(**inputs)
    res = run_bass_kernel_spmd(nc, in_maps, core_ids=list(range(NCORES)),
                               trace=trace)
    outs = [r["out"] for r in res.results]                  # each [NCLS, BC]
    full = np.concatenate([o.T for o in outs], axis=0)      # [B, NCLS]
    return full.astype(np.float32), res


def kernel(**inputs):
    out, _ = run(inputs, trace=False)
    return out


# revision 13
# speedup vs baseline: 1.3105x; 1.1433x over previous
"""Trainium2 Bass kernel for nn_CheckEmotion: embedding gather -> LSTM(128) -> linear(28).

Strategy (data-parallel over 8 NeuronCores, 64 batch rows each):
  - Embedding gather via ONE batched indirect DMA per 8192-token block
    (multi-column offset AP), table padded to 68 cols with col 64 == 1.0
    (the ones column folds the gate bias into the input projection matmul).
  - ONE blocked DMA-xbar transpose per block via 3D APs:
    xeT[e, j, t] = gath[t, j, e].
  - Input projections gx per 8-step window directly into PSUM
    (K=68 bf16 matmul, N=512; bias comes from the ones row).
  - Recurrence: per step 4 gate matmuls (bf16, K=128, N=64) accumulate onto
    gx in PSUM; sigmoid over [i,f,o] banks in one ACT op, tanh for g;
    fused [sig_i|sig_f] * [tanh_g|c_prev] DVE mul; h kept in bf16.
  - Final linear computed transposed ([28, 64] = w_lin @ h.T + b); host
    transposes back.
"""
import sys
import numpy as np
from contextlib import ExitStack

sys.path.insert(0, '/opt/trn_rl_repo')

import concourse.bass as bass
import concourse.tile as tile
from concourse import bacc, mybir
from concourse.bass_utils import run_bass_kernel_spmd

V, E, H, NCLS = 50257, 64, 128, 28
B, T = 512, 1024
NCORES = 8
BC = B // NCORES            # 64 batch rows per core
TOK = BC * T                # 65536 tokens per core
BLK = 8192                  # tokens per gather block
NBLK = TOK // BLK           # 8
WSTEPS = 8                  # steps per psum window
TPB = BLK // 128            # gather tiles per block (64)

F32 = mybir.dt.float32
BF16 = mybir.dt.bfloat16
I32 = mybir.dt.int32
EP = 128                    # padded embedding row: 64 emb + ones col (bias) + zeros

_NC_CACHE = {}


def build_nc():
    if 'nc' in _NC_CACHE:
        return _NC_CACHE['nc']
    nc = bacc.Bacc("TRN2", target_bir_lowering=False, debug=False)
    emb = nc.dram_tensor("emb", [V, EP], BF16, kind="ExternalInput")
    idx = nc.dram_tensor("idx", [128, TOK // 128], I32, kind="ExternalInput")
    wih = nc.dram_tensor("wih", [EP, 4 * H], BF16, kind="ExternalInput")  # lhsT, gate-major i,f,o,g; row 64 = bias
    whh = nc.dram_tensor("whh", [H, 4 * H], BF16, kind="ExternalInput")   # lhsT, gate-major i,f,o,g
    wlin = nc.dram_tensor("wlin", [H, NCLS], F32, kind="ExternalInput")   # lhsT
    blin = nc.dram_tensor("blin", [NCLS, 1], F32, kind="ExternalInput")
    out = nc.dram_tensor("out", [NCLS, BC], F32, kind="ExternalOutput")
    dbg_x = nc.dram_tensor("dbg_x", [EP, 256], BF16, kind="ExternalOutput")

    with tile.TileContext(nc) as tc, ExitStack() as ctx:
        singles = ctx.enter_context(tc.tile_pool(name="singles", bufs=1))
        gathp = ctx.enter_context(tc.tile_pool(name="gath", bufs=2))
        xep = ctx.enter_context(tc.tile_pool(name="xeT", bufs=2))
        psump = ctx.enter_context(tc.tile_pool(name="ps", bufs=2, space="PSUM"))
        gatep = ctx.enter_context(tc.tile_pool(name="gates", bufs=3))
        tmpp = ctx.enter_context(tc.tile_pool(name="tmp", bufs=4))
        statep = ctx.enter_context(tc.tile_pool(name="state", bufs=3))

        idx_sb = singles.tile([128, TOK // 128], I32)
        nc.sync.dma_start(out=idx_sb[:], in_=idx[:, :])
        wih_sb = singles.tile([EP, 4 * H], BF16)
        nc.sync.dma_start(out=wih_sb[:], in_=wih[:, :])
        whh_sb = singles.tile([H, 4 * H], BF16)
        nc.sync.dma_start(out=whh_sb[:], in_=whh[:, :])
        wlin_sb = singles.tile([H, NCLS], F32)
        nc.sync.dma_start(out=wlin_sb[:], in_=wlin[:, :])
        blin_sb = singles.tile([NCLS, 1], F32)
        nc.sync.dma_start(out=blin_sb[:], in_=blin[:, :])

        h_prev = statep.tile([H, BC], BF16, tag="h")
        nc.vector.memset(h_prev[:], 0.0)
        # cpair_j holds [tanh(g_j) | c_{j-1}] for the fused pair multiply.
        cpair_cur = statep.tile([H, 2, BC], F32, tag="cpair")
        nc.vector.memset(cpair_cur[:, 1, :], 0.0)

        for blk in range(NBLK):
            gath = gathp.tile([128, TPB, EP], BF16, tag="gath")
            for j in range(TPB):
                nc.gpsimd.indirect_dma_start(
                    out=gath[:, j, :],
                    out_offset=None,
                    in_=emb[:, :],
                    in_offset=bass.IndirectOffsetOnAxis(
                        ap=idx_sb[:, blk * TPB + j:blk * TPB + j + 1], axis=0),
                )
            # Blocked xbar transpose: xeT[e, j, t] = gath[t, j, e]
            xeT = xep.tile([EP, TPB, 128], BF16, tag="xeT")
            nc.sync.dma_start(
                out=xeT[:, :, :],
                in_=gath[:, :, :],
                transpose=True,
            )
            xeT2 = xeT.rearrange("e j t -> e (j t)")
            if blk == 0:
                nc.sync.dma_start(out=dbg_x[:, :], in_=xeT2[:, 0:256])
            for w in range(BLK // (WSTEPS * BC)):            # 16 windows per block
                ps = psump.tile([128, 3, 512], F32, tag="ps")
                psg = psump.tile([128, 512], F32, tag="psg")
                nc.tensor.matmul(
                    out=psg[:, :],
                    lhsT=wih_sb[:, 3 * H:4 * H],
                    rhs=xeT2[:, w * 512:(w + 1) * 512],
                    start=True, stop=False, skip_group_check=True,
                )
                for g in range(3):
                    nc.tensor.matmul(
                        out=ps[:, g, :],
                        lhsT=wih_sb[:, g * H:(g + 1) * H],
                        rhs=xeT2[:, w * 512:(w + 1) * 512],
                        start=True, stop=False, skip_group_check=True,
                    )
                for j in range(WSTEPS):
                    cj = slice(j * BC, (j + 1) * BC)
                    # g-gate matmul FIRST (own psum tile) so its tanh only
                    # waits on this matmul and overlaps the i/f/o matmuls
                    nc.tensor.matmul(
                        out=psg[:, cj],
                        lhsT=whh_sb[:, 3 * H:4 * H],
                        rhs=h_prev[:, :],
                        start=False, stop=True, skip_group_check=True,
                    )
                    for g in range(3):
                        nc.tensor.matmul(
                            out=ps[:, g, cj],
                            lhsT=whh_sb[:, g * H:(g + 1) * H],
                            rhs=h_prev[:, :],
                            start=False, stop=True, skip_group_check=True,
                        )
                    nc.scalar.activation(
                        out=cpair_cur[:, 0, :],
                        in_=psg[:, cj],
                        func=mybir.ActivationFunctionType.Tanh,
                    )
                    gsig = gatep.tile([H, 3, BC], F32, tag="gsig")
                    nc.scalar.activation(
                        out=gsig[:, :, :],
                        in_=ps[:, 0:3, cj],
                        func=mybir.ActivationFunctionType.Sigmoid,
                    )
                    # t12 = [sig_i | sig_f] * [tanh_g | c_prev]
                    t12 = tmpp.tile([H, 2, BC], F32, tag="t12")
                    nc.vector.tensor_mul(t12[:, :, :], gsig[:, 0:2, :],
                                         cpair_cur[:, :, :])
                    cpair_next = statep.tile([H, 2, BC], F32, tag="cpair")
                    nc.vector.tensor_add(cpair_next[:, 1, :], t12[:, 0, :],
                                         t12[:, 1, :])
                    tc_t = tmpp.tile([H, BC], F32, tag="tc")
                    nc.scalar.activation(
                        out=tc_t[:], in_=cpair_next[:, 1, :],
                        func=mybir.ActivationFunctionType.Tanh,
                    )
                    h_new = statep.tile([H, BC], BF16, tag="h")
                    nc.vector.tensor_mul(h_new[:], gsig[:, 2, :], tc_t[:])
                    h_prev = h_new
                    cpair_cur = cpair_next

        ps_f = psump.tile([NCLS, BC], F32, tag="ps")
        wlin_bf = tmpp.tile([H, NCLS], BF16, tag="wlinb")
        nc.scalar.copy(out=wlin_bf[:], in_=wlin_sb[:])
        nc.tensor.matmul(out=ps_f[:, :], lhsT=wlin_bf[:, :], rhs=h_prev[:, :],
                         start=True, stop=True, skip_group_check=True)
        out_sb = tmpp.tile([NCLS, BC], F32, tag="outsb")
        nc.scalar.activation(
            out=out_sb[:, :], in_=ps_f[:, :],
            func=mybir.ActivationFunctionType.Identity,
            bias=blin_sb[:, :],
        )
        nc.sync.dma_start(out=out[:, :], in_=out_sb[:, :])

    nc.compile()
    _NC_CACHE['nc'] = nc
    return nc


def prep_inputs(x, emb_table, w_ih, w_hh, b_ih, b_hh, w_lin, b_lin):
    """Host-side prep: shard batch, reorder weights gate-major [i,f,o,g] as lhsT."""
    import ml_dtypes
    x = np.asarray(x)
    emb_f32 = np.asarray(emb_table, dtype=np.float32)
    emb_pad = np.zeros((V, EP), dtype=ml_dtypes.bfloat16)
    emb_pad[:, :E] = emb_f32.astype(ml_dtypes.bfloat16)
    emb_pad[:, E] = 1.0                                     # ones col -> bias row of wih
    w_ih = np.asarray(w_ih, dtype=np.float32)
    w_hh = np.asarray(w_hh, dtype=np.float32)
    bias = np.asarray(b_ih, dtype=np.float32) + np.asarray(b_hh, dtype=np.float32)
    w_lin = np.asarray(w_lin, dtype=np.float32)
    b_lin = np.asarray(b_lin, dtype=np.float32)

    # torch gate order i,f,g,o -> our psum order i,f,o,g
    perm = [0, 1, 3, 2]
    wih_g = w_ih.reshape(4, H, E)[perm]                     # [4, H, E]
    whh_g = w_hh.reshape(4, H, H)[perm]
    bias_g = bias.reshape(4, H)[perm]
    wih_lhsT = np.zeros((EP, 4 * H), dtype=ml_dtypes.bfloat16)
    wih_lhsT[:E] = np.ascontiguousarray(
        np.transpose(wih_g, (2, 0, 1)).reshape(E, 4 * H)).astype(
            ml_dtypes.bfloat16)                             # [E, 4H] (gate-major cols)
    wih_lhsT[E] = bias_g.reshape(4 * H).astype(ml_dtypes.bfloat16)
    whh_lhsT = np.ascontiguousarray(
        np.transpose(whh_g, (2, 0, 1)).reshape(H, 4 * H)).astype(
            ml_dtypes.bfloat16)
    wlin_lhsT = np.ascontiguousarray(w_lin.T)               # [H, NCLS]
    blin = np.ascontiguousarray(b_lin.reshape(NCLS, 1))

    in_maps = []
    for c in range(NCORES):
        xc = x[c * BC:(c + 1) * BC]                         # [BC, T]
        toks = np.ascontiguousarray(xc.T).reshape(-1)       # t-major: k = t*BC + b
        idx_host = np.ascontiguousarray(
            toks.astype(np.int32).reshape(TOK // 128, 128).T)  # [128, TOK/128]
        in_maps.append({
            "emb": emb_pad,
            "idx": idx_host,
            "wih": wih_lhsT,
            "whh": whh_lhsT,
            "wlin": wlin_lhsT,
            "blin": blin,
        })
    return in_maps


def run(inputs, trace=False):
    nc = build_nc()
    in_maps = prep_inputs(**inputs)
    res = run_bass_kernel_spmd(nc, in_maps, core_ids=list(range(NCORES)),
                               trace=trace)
    outs = [r["out"] for r in res.results]                  # each [NCLS, BC]
    full = np.concatenate([o.T for o in outs], axis=0)      # [B, NCLS]
    return full.astype(np.float32), res


def kernel(**inputs):
    out, _ = run(inputs, trace=False)
    return out
